# revision 54
# baseline (speedup 1.0000x reference)
"""Trainium2 8-core kernel for multi-head cross-attention — head-parallel.

Problem: B=2, N=M=2048, C=1024, H=8 heads, DH=128.
  q = xq @ Wq + bq ; k = xkv @ Wk + bk ; v = xkv @ Wv + bv
  out = softmax(q k^T / sqrt(DH)) v @ Wo + bo

Sharding (Megatron-style, per the hint): data-parallel over batch across the
two 4-core groups; within a group, tensor-parallel over heads. Core c owns
batch b=c//4 and heads {2t, 2t+1} with t=c%4: it computes k/v projections
for its two heads over ALL 2048 tokens (k/v never leave SBUF — no kv
gather/reload), runs full attention for those heads with the q projection
software-pipelined TWO chunks ahead into the attention loop, applies its
two-head slice of the output projection over all tokens (bo is added
host-side after the gather), and a fp16 ReduceScatter(add) sums the four
partials while scattering partition-row quarters (the partial buffer is
partition-major so every DMA is long contiguous runs; the host remaps rows
to tokens). The reduce is fire-and-forget: nothing in-rep waits on it.
Input loads issue from the idle SP queue (HWDGE), pre-tiled on the host to
the exact SBUF layout (128 contiguous descriptors per tensor).

Compute dtype: fp16 operands, fp32 PSUM accumulation. Activations are kept
feature-major (x^T, q^T, k^T, ctx^T) so contractions land on SBUF
partitions; v is token-major, exactly the stationary layout the ctx matmul
needs. Softmax: transposed scores S^T[tk,tq], exp on ACT; the E-block sums
accumulate INCREMENTALLY on DVE as each exp chunk lands; the denominator
matmul uses an all-ones [128,128] stationary so it broadcasts den to every
partition in the same (128,128) PE tile config as S/ctx (a [1,x]-operand
matmul forces a PE tile reconfig costing ~4 us each on HW), and DVE
reciprocal of the [128,512] result directly yields the broadcast 1/den.
norm work for iter i runs at the head of iter i+1; only the final multiply
remains at the tail. outproj slabs 0..2 are emitted before the last-chunk
ctx drain (their norm_muls completed in-loop), keeping the PE busy while
ACT/DVE finish the last iter. No max subtraction (scores ~N(0,1), safe for
exp).
"""

import sys

for _p in ("/opt/trn_rl_repo",):
    if _p not in sys.path:
        sys.path.insert(0, _p)

import numpy as np

import bass_rust
import concourse.bass as bass
import concourse.mybir as mybir
import concourse.tile as tile
from concourse.bass_utils import run_bass_kernel_spmd

B, N, C, H, DH = 2, 2048, 1024, 8, 128
NCORES, G = 8, 4
CHUNK = N // G  # output tokens per core
KT = C // 128  # 128-wide channel tiles
NJ = N // 128  # kv token tiles
HL = 2  # heads per core
HC = HL * DH  # head channels per core (256)
SCALE = 1.0 / float(np.sqrt(DH))
F16, F32 = mybir.dt.float16, mybir.dt.float32
AF = mybir.ActivationFunctionType
OUT_NP_DTYPE = np.float16
# output stays partition-major end-to-end: partial[p, (tq, tb4, c)], the
# ReduceScatter byte-splits it into 32-partition-row quarters, and the host
# maps row (pl, tq, tb4) of its quarter to token tq*512 + tb4*128 + t*32 + pl
OUT_SHAPE = (128 // G, G * 4 * C)


def _split_excess_waits(nc):
    """Hoist semaphore waits beyond the walrus per-instruction cap onto
    NoOps on the same engine queue (same workaround as kernel.py)."""
    seq = [0]
    for f in nc.m.functions:
        for bb in f.blocks:
            out = []
            for ins in bb.instructions:
                si = ins.sync_info
                if si is None:
                    out.append(ins)
                    continue
                waits = list(si.on_wait)
                cap = 2 if isinstance(ins, mybir.InstEventSemaphore) else 1
                if len(waits) > cap and ins.engine != mybir.EngineType.Unassigned:
                    movable = [w for w in waits if w.sync_type == "semaphore"]
                    keep = [w for w in waits if w.sync_type != "semaphore"]
                    nkeep = cap - len(keep)
                    assert nkeep >= 0, f"{ins.name}: non-sem waits exceed cap"
                    if nkeep > 0:
                        keep += movable[-nkeep:]
                        movable = movable[:-nkeep]
                    for w in movable:
                        seq[0] += 1
                        nop = mybir.InstNoOp(
                            name=f"wsplit_{seq[0]}_{ins.name}", ins=[], outs=[])
                        nop.engine = ins.engine
                        nop.sync_info = bass_rust.SyncInfo(
                            on_wait=[w], on_update=[])
                        out.append(nop)
                    ins.sync_info = bass_rust.SyncInfo(
                        on_wait=keep, on_update=list(si.on_update))
                out.append(ins)
            bb.instructions = out


DEFAULT_OPTS = {
    "dma_on_sync": False,  # issue DMAs from SP (HWDGE) instead of Pool
    "sim_nocoll": False,   # local reduce stand-in instead of ReduceScatter
                           # (TimelineSim is single-core, no collectives)
    "loads_on_sp": True,   # REQUIRED: loads are emitted after the previous
                           # rep's ReduceScatter, so on the Pool queue they
                           # would serialize behind it; SP keeps them free
    "qproj_il": True,      # emit qproj chunk (i+2) inside attention iter i
    "outproj_il": False,   # emit outproj slab u inside attention iter (1,u+1)
    "qproj_eng": "dve",    # engine for the in-loop qproj bias+move
    "outproj_split": True, # alternate outproj PSUM->SBUF moves ACT/DVE
    "psa_bufs": 2,         # PSUM bufs for qproj/den tiles
    "psc_bufs": 2,         # PSUM bufs for ctx accumulators
    "upto": "all",         # phase stripping for profiling: proj|attn|all
    "aux_lite": False,     # profiling: skip Esum/norm chain (wrong output)
    "aux_level": 4,        # profiling bisect: 1=+acc 2=+den/recip 3=+bcast
}


def _resolve_opts(opts: dict | None = None):
    import json as _json
    import os as _os
    env = _json.loads(_os.environ.get("KOPTS2", "{}"))
    return {**DEFAULT_OPTS, **env, **(opts or {})}


def build_nc(reps: int = 1, opts: dict | None = None):
    opts = _resolve_opts(opts)
    nc = bass.Bass("TRN2", target_bir_lowering=False, debug=False,
                   num_devices=NCORES)

    # all bulk inputs arrive pre-tiled to the exact SBUF layout (row p =
    # partition p's contiguous data) so every load is 128 large contiguous
    # descriptors instead of 1024 small strided ones
    ap = {}
    for name, shape, dt in [
        ("xqT", [128, KT * N], F16),
        ("xkvT", [128, KT * N], F16),
        ("wq", [128, KT * HC], F16),
        ("wk", [128, KT * HC], F16),
        ("wv", [128, KT * HC], F16),
        ("wo", [128, HL * C], F16),
        ("bq_col", [128, HL], F32),
        ("bk_col", [128, HL], F32),
        ("bv_row", [128, 2 * HC], F32),
        ("ones_mat", [128, 128], F16),
    ]:
        ap[name] = nc.dram_tensor(name, shape, dt, kind="ExternalInput").ap()
    out_ap = nc.dram_tensor("out", list(OUT_SHAPE), F16,
                            kind="ExternalOutput").ap()

    with tile.TileContext(nc) as tc:
        with (
            tc.tile_pool(name="const", bufs=1) as pconst,
            tc.tile_pool(name="w", bufs=1) as pw,
            tc.tile_pool(name="xT", bufs=1) as pxT,
            tc.tile_pool(name="acts", bufs=1) as pact,
            tc.tile_pool(name="E", bufs=3) as pE,
            tc.tile_pool(name="small", bufs=2) as psmall,
            tc.tile_pool(name="outp", bufs=2) as pout,
            tc.tile_pool(name="psA", bufs=opts["psa_bufs"],
                         space="PSUM") as psA,
            tc.tile_pool(name="psS", bufs=2, space="PSUM") as psS,
            tc.tile_pool(name="psC", bufs=opts["psc_bufs"],
                         space="PSUM") as psC,
            tc.tile_pool(name="dram", bufs=1, space="DRAM") as pdram,
        ):

            pools = (pconst, pw, pxT, pact, pE, psmall, pout,
                     psA, psS, psC, pdram)
            # software-pipelined across reps: loads for rep r are emitted at
            # the end of rep r-1's body (prologue covers rep 0), so they
            # overlap the previous rep's ACT-bound attention phase. Tile's
            # WAR tracking delays each load until its tile's last reader.
            tiles = _emit_loads(nc, ap, pools, opts, first=True)
            for _ in range(reps):
                _emit_compute(nc, ap, out_ap, pools, opts, tiles)
                tiles = _emit_loads(nc, ap, pools, opts, first=False)
    _split_excess_waits(nc)
    return nc


def _emit_loads(nc, ap, pools, opts, first):
    (pconst, pw, pxT, pact, pE, psmall, pout,
     psA, psS, psC, pdram) = pools
    dma = nc.sync.dma_start if opts["dma_on_sync"] else nc.gpsimd.dma_start
    dma_in = nc.sync.dma_start if (opts["loads_on_sp"]
                                   or opts["dma_on_sync"]) else dma

    ones_m = pconst.tile([128, 128], F16, tag="ones_m", name="ones_m")
    dma_in(ones_m[:], ap["ones_mat"])
    bq_sb = pconst.tile([128, HL], F32, tag="bq_sb", name="bq_sb")
    dma_in(bq_sb[:], ap["bq_col"])
    bk_sb = pconst.tile([128, HL], F32, tag="bk_sb", name="bk_sb")
    dma_in(bk_sb[:], ap["bk_col"])
    bv_sb = pconst.tile([128, 2 * HC], F32, tag="bv_sb", name="bv_sb")
    dma_in(bv_sb[:], ap["bv_row"])

    if first:
        # Preload the exp ACT table while input DMAs run.
        dummy = psmall.tile([1, 8], F32, tag="dummy", name="dummy")
        nc.scalar.activation(dummy[:], ones_m[0:1, 0:8], AF.Exp)

    def load_w(name, cols, kt=KT):
        t = pw.tile([128, kt * cols], F16, tag=name, name=name)
        dma_in(t[:], ap[name])
        return t

    # wk + xkv first: kproj is the first PE consumer
    wk_sb = load_w("wk", HC)
    xkvT_sb = pxT.tile([128, KT * N], F16, tag="xkvT", name="xkvT")
    dma_in(xkvT_sb[:], ap["xkvT"])
    wv_sb = load_w("wv", HC)
    wq_sb = load_w("wq", HC)
    xqT_sb = pxT.tile([128, KT * N], F16, tag="xqT", name="xqT")
    dma_in(xqT_sb[:], ap["xqT"])
    wo_sb = load_w("wo", C, kt=HL)
    return dict(ones_m=ones_m, bq_sb=bq_sb, bk_sb=bk_sb,
                bv_sb=bv_sb, wk_sb=wk_sb, wv_sb=wv_sb,
                wq_sb=wq_sb, wo_sb=wo_sb, xkvT_sb=xkvT_sb, xqT_sb=xqT_sb)


def _emit_compute(nc, ap, out_ap, pools, opts, tiles):
    (pconst, pw, pxT, pact, pE, psmall, pout,
     psA, psS, psC, pdram) = pools
    dma = nc.sync.dma_start if opts["dma_on_sync"] else nc.gpsimd.dma_start
    ones_m = tiles["ones_m"]
    bq_sb = tiles["bq_sb"]
    bk_sb = tiles["bk_sb"]
    bv_sb = tiles["bv_sb"]
    wk_sb = tiles["wk_sb"]
    wv_sb = tiles["wv_sb"]
    wq_sb = tiles["wq_sb"]
    wo_sb = tiles["wo_sb"]
    xkvT_sb = tiles["xkvT_sb"]
    xqT_sb = tiles["xqT_sb"]

    partial = pdram.tile([128, G * 4 * C], F16, tag="partial",
                         name="partial")
    rs_out = pdram.tile(list(OUT_SHAPE), F16, tag="rs_out", name="rs_out")
    rg = [[0, 1, 2, 3], [4, 5, 6, 7]]

    # k^T / q^T projections: dst cols (hl, tok)
    kT_sb = pact.tile([128, HL * N], F16, tag="kT_sb", name="kT_sb")
    qT_sb = pact.tile([128, HL * N], F16, tag="qT_sb", name="qT_sb")

    def kqproj(w_sb, b_sb, x_sb, dst, m, tc, eng="act"):
        ps = psA.tile([128, 512], F32, tag="ps", name="ps")
        for k in range(KT):
            nc.tensor.matmul(ps[:],
                             w_sb[:, k * HC + m * 128:k * HC + (m + 1) * 128],
                             x_sb[:, k * N + tc * 512:k * N + (tc + 1) * 512],
                             start=(k == 0), stop=(k == KT - 1))
        dsl = dst[:, m * N + tc * 512:m * N + (tc + 1) * 512]
        if eng == "act":
            nc.scalar.activation(dsl, ps[:], AF.Identity, bias=b_sb[:, m:m + 1])
        else:
            # bias+move on DVE: keeps ACT free for exp during the
            # attention loop (ACT is the loop's bottleneck engine)
            nc.vector.tensor_scalar_add(dsl, ps[:], b_sb[:, m:m + 1])

    # v projection, token-major: v_sb cols (j, hl*128+c); two j-blocks share
    # one PSUM bank so the bias add is 512 wide
    v_sb = pact.tile([128, NJ * HC], F16, tag="v_sb", name="v_sb")

    def vproj(jj):
        ps = psA.tile([128, 512], F32, tag="ps", name="ps")
        for u in range(2):
            j = jj * 2 + u
            for k in range(KT):
                nc.tensor.matmul(
                    ps[:, u * HC:(u + 1) * HC],
                    xkvT_sb[:, k * N + j * 128:k * N + (j + 1) * 128],
                    wv_sb[:, k * HC:(k + 1) * HC],
                    start=(k == 0), stop=(k == KT - 1))
        nc.vector.tensor_add(v_sb[:, jj * 512:(jj + 1) * 512], ps[:],
                             bv_sb[:])

    for m in range(HL):
        for tc in range(G):
            kqproj(wk_sb, bk_sb, xkvT_sb, kT_sb, m, tc)
    for jj in range(NJ // 2):
        vproj(jj)
    if opts["qproj_il"] and opts["upto"] != "proj":
        # two chunks ahead: S(i) never waits on the same-iter qproj move
        kqproj(wq_sb, bq_sb, xqT_sb, qT_sb, 0, 0)
        kqproj(wq_sb, bq_sb, xqT_sb, qT_sb, 0, 1)
    else:
        for m in range(HL):
            for tc in range(G):
                kqproj(wq_sb, bq_sb, xqT_sb, qT_sb, m, tc)
    if opts["upto"] == "proj":
        dma(out_ap[0:128, 0:512], kT_sb[:, 0:512])
        return

    ctxT_sb = pact.tile([128, HL * N], F16, tag="ctxT_sb", name="ctxT_sb")

    # softmax denominator handling, pipelined to minimize tail latency:
    # the per-key-block partial sums of E accumulate incrementally on DVE
    # as each exp chunk lands (instead of one serial reduction after the
    # whole iter), and the denominator/reciprocal/broadcast for iter i run
    # at the HEAD of iter i+1 so only the final multiply remains at the
    # tail. This cut ~6 us/iter of serial aux-chain latency on HW.
    def norm_head(pv):
        # denominator matmul with an all-ones [128,128] STATIONARY: every
        # output partition gets the same column sum, so the matmul itself
        # performs the partition broadcast. Keeps the PE in the same
        # (128,128) tile config as S/ctx — the earlier [1,x]-operand
        # den/broadcast matmuls forced a tile reconfig costing ~4 us EACH
        # on HW. The [128,512] reciprocal costs the same as [1,512] on DVE
        # (per-partition lanes) and directly yields the broadcast.
        if opts["aux_lite"] or opts["aux_level"] < 2:
            return None
        denp = psA.tile([128, 512], F32, tag="ps", name="den")
        nc.tensor.matmul(denp[:], ones_m[:], pv["Esum"][:, 0:512],
                         start=True, stop=True)
        bcast_sb = psmall.tile([128, 512], F16, tag="bcast", name="bcast")
        with nc.allow_low_precision("softmax denom recip in f16; tol 2e-2"):
            nc.vector.reciprocal(bcast_sb[:], denp[:])
        return bcast_sb

    def norm_mul(pv, ctxp, bcast_sb):
        dsl = ctxT_sb[:, pv["hl"] * N + pv["u"] * 512
                      :pv["hl"] * N + (pv["u"] + 1) * 512]
        if opts["aux_lite"] or opts["aux_level"] < 4:
            nc.vector.tensor_copy(dsl, ctxp[:])
            return
        nc.vector.tensor_mul(dsl, ctxp[:], bcast_sb[:])

    # attention: 8 iterations of (head hl, query chunk u), software-pipelined
    # so ctx(i-1) matmuls interleave with S/exp(i)
    def ctx_mm(pctx, phl, pE_, j):
        nc.tensor.matmul(
            pctx[:], v_sb[:, j * HC + phl * 128:j * HC + (phl + 1) * 128],
            pE_[:, j * 512:(j + 1) * 512], start=(j == 0), stop=(j == NJ - 1))

    def outproj_slab(tq):
        og = pout.tile([128, 4 * C], F16, tag="og", name="og")
        for tb4 in range(4):
            tb = tq * 4 + tb4
            po = psS.tile([128, 1024], F32, tag="S", name="S")
            for n in range(2):
                for hl2 in range(HL):
                    nc.tensor.matmul(
                        po[:, n * 512:(n + 1) * 512],
                        ctxT_sb[:, hl2 * N + tb * 128:hl2 * N + (tb + 1) * 128],
                        wo_sb[:, hl2 * C + n * 512:hl2 * C + (n + 1) * 512],
                        start=(hl2 == 0), stop=(hl2 == HL - 1))
            # bo is added host-side after the gather; the PSUM->SBUF
            # moves alternate ACT/DVE so neither engine serializes the
            # output phase
            osl = og[:, tb4 * C:(tb4 + 1) * C]
            if opts["outproj_split"] and tb4 % 2 == 0:
                nc.scalar.activation(osl, po[:], AF.Identity)
            else:
                nc.vector.tensor_copy(osl, po[:])
        dma(partial[:, tq * 4 * C:(tq + 1) * 4 * C], og[:])

    iters = [(hl, u) for hl in range(HL) for u in range(G)]
    prev = None
    for it, (hl, u) in enumerate(iters):
        qslice = qT_sb[:, hl * N + u * 512:hl * N + (u + 1) * 512]
        E = pE.tile([128, NJ * 512], F16, tag="E", name="E")
        acc = bcast_sb = None
        if prev is not None:
            pctx = psC.tile([128, 512], F32, tag="ctx", name="ctx")
            bcast_sb = norm_head(prev)
        for jj in range(NJ // 2):
            Sp = psS.tile([128, 1024], F32, tag="S", name="S")
            for w in range(2):
                j = jj * 2 + w
                nc.tensor.matmul(
                    Sp[:, w * 512:(w + 1) * 512],
                    kT_sb[:, hl * N + j * 128:hl * N + (j + 1) * 128],
                    qslice, start=True, stop=True)
            nc.scalar.activation(E[:, jj * 1024:(jj + 1) * 1024], Sp[:],
                                 AF.Exp, scale=SCALE)
            if not opts["aux_lite"] and opts["aux_level"] >= 1:
                if jj == 1:
                    acc = psmall.tile([128, 1024], F16, tag="eacc",
                                      name="eacc")
                    nc.vector.tensor_add(acc[:], E[:, 0:1024],
                                         E[:, 1024:2048])
                elif jj >= 2:
                    nc.vector.tensor_add(acc[:], acc[:],
                                         E[:, jj * 1024:(jj + 1) * 1024])
            if prev is not None:
                for j in (jj * 2, jj * 2 + 1):
                    ctx_mm(pctx, prev["hl"], prev["E"], j)
        if prev is not None:
            norm_mul(prev, pctx, bcast_sb)
            if (opts["outproj_il"] and prev["hl"] == 1
                    and prev["u"] < G - 1):
                outproj_slab(prev["u"])
        if opts["qproj_il"] and it + 2 < len(iters):
            nhl, nu = iters[it + 2]
            kqproj(wq_sb, bq_sb, xqT_sb, qT_sb, nhl, nu,
                   eng=opts["qproj_eng"])
        if not opts["aux_lite"] and opts["aux_level"] >= 1:
            nc.vector.tensor_add(acc[:, 0:512], acc[:, 0:512],
                                 acc[:, 512:1024])
        prev = {"hl": hl, "u": u, "E": E, "Esum": acc}
    # outproj slab tq only needs norm_mul(1,tq): slabs 0..2 were satisfied
    # in-loop, so emit them FIRST — they keep the PE busy while ACT/DVE
    # finish the last iter's exp/acc/fold — then drain the last chunk and
    # emit slab 3
    if opts["upto"] != "attn" and not opts["outproj_il"]:
        for tq in range(G - 1):
            outproj_slab(tq)
    pctx = psC.tile([128, 512], F32, tag="ctx", name="ctx")
    for j in range(NJ):
        ctx_mm(pctx, prev["hl"], prev["E"], j)
    bcast_sb = norm_head(prev)
    norm_mul(prev, pctx, bcast_sb)
    if opts["upto"] == "attn":
        dma(out_ap[0:128, 0:512], ctxT_sb[:, 0:512])
        return

    # output projection partials over ALL tokens for my two heads:
    # partial[tok, ch] = sum_{hl,dh} ctx^T[hl][dh, tok] wo[(hl,dh), ch]
    outproj_slab(G - 1)

    if opts["sim_nocoll"]:
        dma(rs_out[:], partial[0:OUT_SHAPE[0], :])
    else:
        nc.gpsimd.collective_compute(
            "ReduceScatter", mybir.AluOpType.add, replica_groups=rg,
            ins=[partial.opt()], outs=[rs_out.opt()])
    # fire-and-forget: nothing in-rep waits on the reduce or this copy
    dma(out_ap[:], rs_out[:])



def prep_in_maps(inputs_q, inputs_kv, Wq, bq, Wk, bk, Wv, bv, Wo, bo):
    """Host-side layout prep: per-core head slices, transpose to
    feature-major, fp16 casts, bias layout tiles. No FLOPs beyond casts."""
    inputs_q = np.asarray(inputs_q, dtype=np.float32)
    inputs_kv = np.asarray(inputs_kv, dtype=np.float32)
    Wq = np.asarray(Wq, np.float32)
    Wk = np.asarray(Wk, np.float32)
    Wv = np.asarray(Wv, np.float32)
    Wo = np.asarray(Wo, np.float32)
    bq = np.asarray(bq, np.float32)
    bk = np.asarray(bk, np.float32)
    bv = np.asarray(bv, np.float32)
    bo = np.asarray(bo, np.float32)
    def ptile(m, kt):
        # [kt*128, cols] -> [128, kt*cols]: row p holds partition p's
        # SBUF data contiguously (k-tile-major), matching the kernel's
        # on-chip layout so each DMA descriptor is one long run
        cols = m.shape[1]
        return np.ascontiguousarray(
            m.reshape(kt, 128, cols).transpose(1, 0, 2).reshape(128, -1)
            .astype(np.float16))

    xT = {}
    for b in range(B):
        xT[("q", b)] = ptile(inputs_q[b].T, KT)
        xT[("kv", b)] = ptile(inputs_kv[b].T, KT)
    shared = {
        "ones_mat": np.ones((128, 128), np.float16),
    }
    in_maps = []
    for c in range(NCORES):
        b, t = divmod(c, G)
        hsl = slice(2 * t * DH, 2 * t * DH + HC)
        bvs = np.tile(bv[hsl], 2)
        in_maps.append({
            "xqT": xT[("q", b)],
            "xkvT": xT[("kv", b)],
            "wq": ptile(Wq[:, hsl], KT),
            "wk": ptile(Wk[:, hsl], KT),
            "wv": ptile(Wv[:, hsl], KT),
            "wo": ptile(Wo[hsl, :], HL),
            "bq_col": np.ascontiguousarray(bq[hsl].reshape(HL, 128).T),
            "bk_col": np.ascontiguousarray(bk[hsl].reshape(HL, 128).T),
            "bv_row": np.ascontiguousarray(np.broadcast_to(bvs, (128, 2 * HC))),
            **shared,
        })
    return in_maps


def kernel(inputs_q, inputs_kv, Wq, bq, Wk, bk, Wv, bv, Wo, bo):
    in_maps = prep_in_maps(inputs_q, inputs_kv, Wq, bq, Wk, bk, Wv, bv, Wo, bo)
    nc = build_nc(reps=1)
    res = run_bass_kernel_spmd(nc, in_maps, core_ids=list(range(NCORES)))
    out = np.empty((B, N, C), np.float32)
    outv = out.reshape(B, G, 4, G, 32, C)  # [b, tq, tb4, t, pl, c]
    for c in range(NCORES):
        b, t = divmod(c, G)
        o = res.results[c]["out"].astype(np.float32)
        # o[pl, (tq, tb4, c)] -> tokens tq*512 + tb4*128 + t*32 + pl
        outv[b, :, :, t] = o.reshape(32, G, 4, C).transpose(1, 2, 0, 3)
    out += np.asarray(bo, np.float32)  # bo applied host-side
    return out


if __name__ == "__main__":
    rng = np.random.default_rng(0)
    s = 1.0 / np.sqrt(C)
    ins = {
        "inputs_q": rng.standard_normal((B, N, C), np.float32),
        "inputs_kv": rng.standard_normal((B, N, C), np.float32),
        "Wq": rng.standard_normal((C, C), np.float32) * s,
        "bq": np.zeros(C, np.float32),
        "Wk": rng.standard_normal((C, C), np.float32) * s,
        "bk": np.zeros(C, np.float32),
        "Wv": rng.standard_normal((C, C), np.float32) * s,
        "bv": np.zeros(C, np.float32),
        "Wo": rng.standard_normal((C, C), np.float32) * s,
        "bo": np.zeros(C, np.float32),
    }
    out = kernel(**ins)
    # numpy reference
    def ref(xq, xkv, Wq, bq, Wk, bk, Wv, bv, Wo, bo):
        q = (xq @ Wq + bq).reshape(B, N, H, DH)
        k = (xkv @ Wk + bk).reshape(B, N, H, DH)
        v = (xkv @ Wv + bv).reshape(B, N, H, DH)
        s_ = np.einsum("bnhc,bmhc->bhnm", q, k) / np.sqrt(DH)
        e = np.exp(s_ - s_.max(-1, keepdims=True))
        p = e / e.sum(-1, keepdims=True)
        o = np.einsum("bhnm,bmhd->bnhd", p, v).reshape(B, N, C)
        return o @ Wo + bo
    exp = ref(ins["inputs_q"], ins["inputs_kv"], ins["Wq"], ins["bq"],
              ins["Wk"], ins["bk"], ins["Wv"], ins["bv"], ins["Wo"],
              ins["bo"])
    err = np.abs(out - exp).max() / np.abs(exp).max()
    print("out", out.shape, out.dtype, "rel err:", err)



# revision 57
# speedup vs baseline: 1.2391x; 1.2391x over previous
"""Trainium2 8-core kernel for multi-head cross-attention — head-parallel.

Problem: B=2, N=M=2048, C=1024, H=8 heads, DH=128.
  q = xq @ Wq + bq ; k = xkv @ Wk + bk ; v = xkv @ Wv + bv
  out = softmax(q k^T / sqrt(DH)) v @ Wo + bo

Sharding (Megatron-style, per the hint): data-parallel over batch across the
two 4-core groups; within a group, tensor-parallel over heads. Core c owns
batch b=c//4 and heads {2t, 2t+1} with t=c%4: it computes k/v projections
for its two heads over ALL 2048 tokens (k/v never leave SBUF — no kv
gather/reload), runs full attention for those heads with the q projection
software-pipelined TWO chunks ahead into the attention loop, applies its
two-head slice of the output projection over all tokens (bo is added
host-side after the gather), and a fp16 ReduceScatter(add) sums the four
partials while scattering partition-row quarters (the partial buffer is
partition-major so every DMA is long contiguous runs; the host remaps rows
to tokens). The reduce is fire-and-forget: nothing in-rep waits on it.
Input loads issue from the idle SP queue (HWDGE), pre-tiled on the host to
the exact SBUF layout (128 contiguous descriptors per tensor).

Compute dtype: fp16 operands, fp32 PSUM accumulation. Activations are kept
feature-major (x^T, q^T, k^T, ctx^T) so contractions land on SBUF
partitions; v is token-major, exactly the stationary layout the ctx matmul
needs. Softmax: transposed scores S^T[tk,tq], exp on ACT; the E-block sums
accumulate INCREMENTALLY on DVE as each exp chunk lands; the denominator
matmul uses an all-ones [128,128] stationary so it broadcasts den to every
partition in the same (128,128) PE tile config as S/ctx (a [1,x]-operand
matmul forces a PE tile reconfig costing ~4 us each on HW), and DVE
reciprocal of the [128,512] result directly yields the broadcast 1/den.
norm work for iter i runs at the head of iter i+1; only the final multiply
remains at the tail. outproj slabs 0..2 are emitted before the last-chunk
ctx drain (their norm_muls completed in-loop), keeping the PE busy while
ACT/DVE finish the last iter. No max subtraction (scores ~N(0,1), safe for
exp).
"""

import sys

for _p in ("/opt/trn_rl_repo",):
    if _p not in sys.path:
        sys.path.insert(0, _p)

import numpy as np

import bass_rust
import concourse.bass as bass
import concourse.mybir as mybir
import concourse.tile as tile
from concourse.bass_utils import run_bass_kernel_spmd

B, N, C, H, DH = 2, 2048, 1024, 8, 128
NCORES, G = 8, 4
CHUNK = N // G  # output tokens per core
KT = C // 128  # 128-wide channel tiles
NJ = N // 128  # kv token tiles
HL = 2  # heads per core
HC = HL * DH  # head channels per core (256)
SCALE = 1.0 / float(np.sqrt(DH))
F16, F32 = mybir.dt.float16, mybir.dt.float32
AF = mybir.ActivationFunctionType
OUT_NP_DTYPE = np.float16
# output stays partition-major end-to-end: partial[p, (tq, tb4, c)], the
# ReduceScatter byte-splits it into 32-partition-row quarters, and the host
# maps row (pl, tq, tb4) of its quarter to token tq*512 + tb4*128 + t*32 + pl
OUT_SHAPE = (128 // G, G * 4 * C)


def _split_excess_waits(nc):
    """Hoist semaphore waits beyond the walrus per-instruction cap onto
    NoOps on the same engine queue (same workaround as kernel.py)."""
    seq = [0]
    for f in nc.m.functions:
        for bb in f.blocks:
            out = []
            for ins in bb.instructions:
                si = ins.sync_info
                if si is None:
                    out.append(ins)
                    continue
                waits = list(si.on_wait)
                cap = 2 if isinstance(ins, mybir.InstEventSemaphore) else 1
                if len(waits) > cap and ins.engine != mybir.EngineType.Unassigned:
                    movable = [w for w in waits if w.sync_type == "semaphore"]
                    keep = [w for w in waits if w.sync_type != "semaphore"]
                    nkeep = cap - len(keep)
                    assert nkeep >= 0, f"{ins.name}: non-sem waits exceed cap"
                    if nkeep > 0:
                        keep += movable[-nkeep:]
                        movable = movable[:-nkeep]
                    for w in movable:
                        seq[0] += 1
                        nop = mybir.InstNoOp(
                            name=f"wsplit_{seq[0]}_{ins.name}", ins=[], outs=[])
                        nop.engine = ins.engine
                        nop.sync_info = bass_rust.SyncInfo(
                            on_wait=[w], on_update=[])
                        out.append(nop)
                    ins.sync_info = bass_rust.SyncInfo(
                        on_wait=keep, on_update=list(si.on_update))
                out.append(ins)
            bb.instructions = out


DEFAULT_OPTS = {
    "dma_on_sync": False,  # issue DMAs from SP (HWDGE) instead of Pool
    "sim_nocoll": False,   # local reduce stand-in instead of ReduceScatter
                           # (TimelineSim is single-core, no collectives)
    "loads_on_sp": True,   # REQUIRED: loads are emitted after the previous
                           # rep's ReduceScatter, so on the Pool queue they
                           # would serialize behind it; SP keeps them free
    "qproj_il": True,      # emit qproj chunk (i+2) inside attention iter i
    "outproj_il": False,   # emit outproj slab u inside attention iter (1,u+1)
    "qproj_eng": "dve",    # engine for the in-loop qproj bias+move
    "outproj_split": True, # alternate outproj PSUM->SBUF moves ACT/DVE
    "psa_bufs": 2,         # PSUM bufs for qproj/den tiles
    "psc_bufs": 2,         # PSUM bufs for ctx accumulators
    "acc512": False,       # 512-wide Esum accumulator (no final fold)
    "upto": "all",         # phase stripping for profiling: proj|attn|all
    "aux_lite": False,     # profiling: skip Esum/norm chain (wrong output)
    "aux_level": 4,        # profiling bisect: 1=+acc 2=+den/recip 3=+bcast
}


def _resolve_opts(opts: dict | None = None):
    import json as _json
    import os as _os
    env = _json.loads(_os.environ.get("KOPTS2", "{}"))
    return {**DEFAULT_OPTS, **env, **(opts or {})}


def build_nc(reps: int = 1, opts: dict | None = None):
    opts = _resolve_opts(opts)
    nc = bass.Bass("TRN2", target_bir_lowering=False, debug=False,
                   num_devices=NCORES)

    # all bulk inputs arrive pre-tiled to the exact SBUF layout (row p =
    # partition p's contiguous data) so every load is 128 large contiguous
    # descriptors instead of 1024 small strided ones
    ap = {}
    for name, shape, dt in [
        ("xqT", [128, KT * N], F16),
        ("xkvT", [128, KT * N], F16),
        ("wq", [128, KT * HC], F16),
        ("wk", [128, KT * HC], F16),
        ("wv", [128, KT * HC], F16),
        ("wo", [128, HL * C], F16),
        ("bq_col", [128, HL], F32),
        ("bk_col", [128, HL], F32),
        ("bv_row", [128, 2 * HC], F32),
        ("ones_mat", [128, 128], F16),
    ]:
        ap[name] = nc.dram_tensor(name, shape, dt, kind="ExternalInput").ap()
    out_ap = nc.dram_tensor("out", list(OUT_SHAPE), F16,
                            kind="ExternalOutput").ap()

    with tile.TileContext(nc) as tc:
        with (
            tc.tile_pool(name="const", bufs=1) as pconst,
            tc.tile_pool(name="w", bufs=1) as pw,
            tc.tile_pool(name="xT", bufs=1) as pxT,
            tc.tile_pool(name="acts", bufs=1) as pact,
            tc.tile_pool(name="E", bufs=3) as pE,
            tc.tile_pool(name="small", bufs=2) as psmall,
            tc.tile_pool(name="outp", bufs=2) as pout,
            tc.tile_pool(name="psA", bufs=opts["psa_bufs"],
                         space="PSUM") as psA,
            tc.tile_pool(name="psS", bufs=2, space="PSUM") as psS,
            tc.tile_pool(name="psC", bufs=opts["psc_bufs"],
                         space="PSUM") as psC,
            tc.tile_pool(name="dram", bufs=1, space="DRAM") as pdram,
        ):

            pools = (pconst, pw, pxT, pact, pE, psmall, pout,
                     psA, psS, psC, pdram)
            # software-pipelined across reps: loads for rep r are emitted at
            # the end of rep r-1's body (prologue covers rep 0), so they
            # overlap the previous rep's ACT-bound attention phase. Tile's
            # WAR tracking delays each load until its tile's last reader.
            tiles = _emit_loads(nc, ap, pools, opts, first=True)
            for _ in range(reps):
                _emit_compute(nc, ap, out_ap, pools, opts, tiles)
                tiles = _emit_loads(nc, ap, pools, opts, first=False)
    _split_excess_waits(nc)
    return nc


def _emit_loads(nc, ap, pools, opts, first):
    (pconst, pw, pxT, pact, pE, psmall, pout,
     psA, psS, psC, pdram) = pools
    dma = nc.sync.dma_start if opts["dma_on_sync"] else nc.gpsimd.dma_start
    dma_in = nc.sync.dma_start if (opts["loads_on_sp"]
                                   or opts["dma_on_sync"]) else dma

    ones_m = pconst.tile([128, 128], F16, tag="ones_m", name="ones_m")
    dma_in(ones_m[:], ap["ones_mat"])
    bq_sb = pconst.tile([128, HL], F32, tag="bq_sb", name="bq_sb")
    dma_in(bq_sb[:], ap["bq_col"])
    bk_sb = pconst.tile([128, HL], F32, tag="bk_sb", name="bk_sb")
    dma_in(bk_sb[:], ap["bk_col"])
    bv_sb = pconst.tile([128, 2 * HC], F32, tag="bv_sb", name="bv_sb")
    dma_in(bv_sb[:], ap["bv_row"])

    if first:
        # Preload the exp ACT table while input DMAs run.
        dummy = psmall.tile([1, 8], F32, tag="dummy", name="dummy")
        nc.scalar.activation(dummy[:], ones_m[0:1, 0:8], AF.Exp)

    def load_w(name, cols, kt=KT):
        t = pw.tile([128, kt * cols], F16, tag=name, name=name)
        dma_in(t[:], ap[name])
        return t

    # wk + xkv first: kproj is the first PE consumer
    wk_sb = load_w("wk", HC)
    xkvT_sb = pxT.tile([128, KT * N], F16, tag="xkvT", name="xkvT")
    dma_in(xkvT_sb[:], ap["xkvT"])
    wv_sb = load_w("wv", HC)
    wq_sb = load_w("wq", HC)
    xqT_sb = pxT.tile([128, KT * N], F16, tag="xqT", name="xqT")
    dma_in(xqT_sb[:], ap["xqT"])
    wo_sb = load_w("wo", C, kt=HL)
    return dict(ones_m=ones_m, bq_sb=bq_sb, bk_sb=bk_sb,
                bv_sb=bv_sb, wk_sb=wk_sb, wv_sb=wv_sb,
                wq_sb=wq_sb, wo_sb=wo_sb, xkvT_sb=xkvT_sb, xqT_sb=xqT_sb)


def _emit_compute(nc, ap, out_ap, pools, opts, tiles):
    (pconst, pw, pxT, pact, pE, psmall, pout,
     psA, psS, psC, pdram) = pools
    dma = nc.sync.dma_start if opts["dma_on_sync"] else nc.gpsimd.dma_start
    ones_m = tiles["ones_m"]
    bq_sb = tiles["bq_sb"]
    bk_sb = tiles["bk_sb"]
    bv_sb = tiles["bv_sb"]
    wk_sb = tiles["wk_sb"]
    wv_sb = tiles["wv_sb"]
    wq_sb = tiles["wq_sb"]
    wo_sb = tiles["wo_sb"]
    xkvT_sb = tiles["xkvT_sb"]
    xqT_sb = tiles["xqT_sb"]

    partial = pdram.tile([128, G * 4 * C], F16, tag="partial",
                         name="partial")
    rs_out = pdram.tile(list(OUT_SHAPE), F16, tag="rs_out", name="rs_out")
    rg = [[0, 1, 2, 3], [4, 5, 6, 7]]

    # k^T / q^T projections: dst cols (hl, tok)
    kT_sb = pact.tile([128, HL * N], F16, tag="kT_sb", name="kT_sb")
    qT_sb = pact.tile([128, HL * N], F16, tag="qT_sb", name="qT_sb")

    def kqproj(w_sb, b_sb, x_sb, dst, m, tc, eng="act"):
        ps = psA.tile([128, 512], F32, tag="ps", name="ps")
        for k in range(KT):
            nc.tensor.matmul(ps[:],
                             w_sb[:, k * HC + m * 128:k * HC + (m + 1) * 128],
                             x_sb[:, k * N + tc * 512:k * N + (tc + 1) * 512],
                             start=(k == 0), stop=(k == KT - 1))
        dsl = dst[:, m * N + tc * 512:m * N + (tc + 1) * 512]
        if eng == "act":
            nc.scalar.activation(dsl, ps[:], AF.Identity, bias=b_sb[:, m:m + 1])
        else:
            # bias+move on DVE: keeps ACT free for exp during the
            # attention loop (ACT is the loop's bottleneck engine)
            nc.vector.tensor_scalar_add(dsl, ps[:], b_sb[:, m:m + 1])

    # v projection, token-major: v_sb cols (j, hl*128+c); two j-blocks share
    # one PSUM bank so the bias add is 512 wide
    v_sb = pact.tile([128, NJ * HC], F16, tag="v_sb", name="v_sb")

    def vproj(jj):
        ps = psA.tile([128, 512], F32, tag="ps", name="ps")
        for u in range(2):
            j = jj * 2 + u
            for k in range(KT):
                nc.tensor.matmul(
                    ps[:, u * HC:(u + 1) * HC],
                    xkvT_sb[:, k * N + j * 128:k * N + (j + 1) * 128],
                    wv_sb[:, k * HC:(k + 1) * HC],
                    start=(k == 0), stop=(k == KT - 1))
        nc.vector.tensor_add(v_sb[:, jj * 512:(jj + 1) * 512], ps[:],
                             bv_sb[:])

    for m in range(HL):
        for tc in range(G):
            kqproj(wk_sb, bk_sb, xkvT_sb, kT_sb, m, tc)
    for jj in range(NJ // 2):
        vproj(jj)
    if opts["qproj_il"] and opts["upto"] != "proj":
        # two chunks ahead: S(i) never waits on the same-iter qproj move
        kqproj(wq_sb, bq_sb, xqT_sb, qT_sb, 0, 0)
        kqproj(wq_sb, bq_sb, xqT_sb, qT_sb, 0, 1)
    else:
        for m in range(HL):
            for tc in range(G):
                kqproj(wq_sb, bq_sb, xqT_sb, qT_sb, m, tc)
    if opts["upto"] == "proj":
        dma(out_ap[0:128, 0:512], kT_sb[:, 0:512])
        return

    ctxT_sb = pact.tile([128, HL * N], F16, tag="ctxT_sb", name="ctxT_sb")

    # softmax denominator handling, pipelined to minimize tail latency:
    # the per-key-block partial sums of E accumulate incrementally on DVE
    # as each exp chunk lands (instead of one serial reduction after the
    # whole iter), and the denominator/reciprocal/broadcast for iter i run
    # at the HEAD of iter i+1 so only the final multiply remains at the
    # tail. This cut ~6 us/iter of serial aux-chain latency on HW.
    def norm_head(pv):
        # denominator matmul with an all-ones [128,128] STATIONARY: every
        # output partition gets the same column sum, so the matmul itself
        # performs the partition broadcast. Keeps the PE in the same
        # (128,128) tile config as S/ctx — the earlier [1,x]-operand
        # den/broadcast matmuls forced a tile reconfig costing ~4 us EACH
        # on HW. The [128,512] reciprocal costs the same as [1,512] on DVE
        # (per-partition lanes) and directly yields the broadcast.
        if opts["aux_lite"] or opts["aux_level"] < 2:
            return None
        denp = psA.tile([128, 512], F32, tag="ps", name="den")
        nc.tensor.matmul(denp[:], ones_m[:], pv["Esum"][:, 0:512],
                         start=True, stop=True)
        bcast_sb = psmall.tile([128, 512], F16, tag="bcast", name="bcast")
        with nc.allow_low_precision("softmax denom recip in f16; tol 2e-2"):
            nc.vector.reciprocal(bcast_sb[:], denp[:])
        return bcast_sb

    def norm_mul(pv, ctxp, bcast_sb):
        dsl = ctxT_sb[:, pv["hl"] * N + pv["u"] * 512
                      :pv["hl"] * N + (pv["u"] + 1) * 512]
        if opts["aux_lite"] or opts["aux_level"] < 4:
            nc.vector.tensor_copy(dsl, ctxp[:])
            return
        nc.vector.tensor_mul(dsl, ctxp[:], bcast_sb[:])

    # attention: 8 iterations of (head hl, query chunk u), software-pipelined
    # so ctx(i-1) matmuls interleave with S/exp(i)
    def ctx_mm(pctx, phl, pE_, j):
        nc.tensor.matmul(
            pctx[:], v_sb[:, j * HC + phl * 128:j * HC + (phl + 1) * 128],
            pE_[:, j * 512:(j + 1) * 512], start=(j == 0), stop=(j == NJ - 1))

    def outproj_slab(tq):
        og = pout.tile([128, 4 * C], F16, tag="og", name="og")
        for tb4 in range(4):
            tb = tq * 4 + tb4
            po = psS.tile([128, 1024], F32, tag="S", name="S")
            for n in range(2):
                for hl2 in range(HL):
                    nc.tensor.matmul(
                        po[:, n * 512:(n + 1) * 512],
                        ctxT_sb[:, hl2 * N + tb * 128:hl2 * N + (tb + 1) * 128],
                        wo_sb[:, hl2 * C + n * 512:hl2 * C + (n + 1) * 512],
                        start=(hl2 == 0), stop=(hl2 == HL - 1))
            # bo is added host-side after the gather; the PSUM->SBUF
            # moves alternate ACT/DVE so neither engine serializes the
            # output phase
            osl = og[:, tb4 * C:(tb4 + 1) * C]
            if opts["outproj_split"] and tb4 % 2 == 0:
                nc.scalar.activation(osl, po[:], AF.Identity)
            else:
                nc.vector.tensor_copy(osl, po[:])
        dma(partial[:, tq * 4 * C:(tq + 1) * 4 * C], og[:])

    iters = [(hl, u) for hl in range(HL) for u in range(G)]
    prev = None
    for it, (hl, u) in enumerate(iters):
        qslice = qT_sb[:, hl * N + u * 512:hl * N + (u + 1) * 512]
        E = pE.tile([128, NJ * 512], F16, tag="E", name="E")
        acc = bcast_sb = None
        if prev is not None:
            pctx = psC.tile([128, 512], F32, tag="ctx", name="ctx")
            bcast_sb = norm_head(prev)
        for jj in range(NJ // 2):
            Sp = psS.tile([128, 1024], F32, tag="S", name="S")
            for w in range(2):
                j = jj * 2 + w
                nc.tensor.matmul(
                    Sp[:, w * 512:(w + 1) * 512],
                    kT_sb[:, hl * N + j * 128:hl * N + (j + 1) * 128],
                    qslice, start=True, stop=True)
            nc.scalar.activation(E[:, jj * 1024:(jj + 1) * 1024], Sp[:],
                                 AF.Exp, scale=SCALE)
            if not opts["aux_lite"] and opts["aux_level"] >= 1:
                if opts["acc512"]:
                    # 512-wide accumulator: 2 adds per exp chunk, but no
                    # final fold on the denominator's critical path
                    if jj == 0:
                        acc = psmall.tile([128, 1024], F16, tag="eacc",
                                          name="eacc")
                        nc.vector.tensor_add(acc[:, 0:512], E[:, 0:512],
                                             E[:, 512:1024])
                    else:
                        for w in range(2):
                            nc.vector.tensor_add(
                                acc[:, 0:512], acc[:, 0:512],
                                E[:, (2 * jj + w) * 512
                                  :(2 * jj + w + 1) * 512])
                elif jj == 1:
                    acc = psmall.tile([128, 1024], F16, tag="eacc",
                                      name="eacc")
                    nc.vector.tensor_add(acc[:], E[:, 0:1024],
                                         E[:, 1024:2048])
                elif jj >= 2:
                    nc.vector.tensor_add(acc[:], acc[:],
                                         E[:, jj * 1024:(jj + 1) * 1024])
            if prev is not None:
                for j in (jj * 2, jj * 2 + 1):
                    ctx_mm(pctx, prev["hl"], prev["E"], j)
        if prev is not None:
            norm_mul(prev, pctx, bcast_sb)
            if (opts["outproj_il"] and prev["hl"] == 1
                    and prev["u"] < G - 1):
                outproj_slab(prev["u"])
        if opts["qproj_il"] and it + 2 < len(iters):
            nhl, nu = iters[it + 2]
            kqproj(wq_sb, bq_sb, xqT_sb, qT_sb, nhl, nu,
                   eng=opts["qproj_eng"])
        if (not opts["aux_lite"] and opts["aux_level"] >= 1
                and not opts["acc512"]):
            nc.vector.tensor_add(acc[:, 0:512], acc[:, 0:512],
                                 acc[:, 512:1024])
        prev = {"hl": hl, "u": u, "E": E, "Esum": acc}
    # outproj slab tq only needs norm_mul(1,tq): slabs 0..2 were satisfied
    # in-loop, so emit them FIRST — they keep the PE busy while ACT/DVE
    # finish the last iter's exp/acc/fold — then drain the last chunk and
    # emit slab 3
    if opts["upto"] != "attn" and not opts["outproj_il"]:
        for tq in range(G - 1):
            outproj_slab(tq)
    pctx = psC.tile([128, 512], F32, tag="ctx", name="ctx")
    for j in range(NJ):
        ctx_mm(pctx, prev["hl"], prev["E"], j)
    bcast_sb = norm_head(prev)
    norm_mul(prev, pctx, bcast_sb)
    if opts["upto"] == "attn":
        dma(out_ap[0:128, 0:512], ctxT_sb[:, 0:512])
        return

    # output projection partials over ALL tokens for my two heads:
    # partial[tok, ch] = sum_{hl,dh} ctx^T[hl][dh, tok] wo[(hl,dh), ch]
    outproj_slab(G - 1)

    if opts["sim_nocoll"]:
        dma(rs_out[:], partial[0:OUT_SHAPE[0], :])
    else:
        nc.gpsimd.collective_compute(
            "ReduceScatter", mybir.AluOpType.add, replica_groups=rg,
            ins=[partial.opt()], outs=[rs_out.opt()])
    # fire-and-forget: nothing in-rep waits on the reduce or this copy
    dma(out_ap[:], rs_out[:])



def prep_in_maps(inputs_q, inputs_kv, Wq, bq, Wk, bk, Wv, bv, Wo, bo):
    """Host-side layout prep: per-core head slices, transpose to
    feature-major, fp16 casts, bias layout tiles. No FLOPs beyond casts."""
    inputs_q = np.asarray(inputs_q, dtype=np.float32)
    inputs_kv = np.asarray(inputs_kv, dtype=np.float32)
    Wq = np.asarray(Wq, np.float32)
    Wk = np.asarray(Wk, np.float32)
    Wv = np.asarray(Wv, np.float32)
    Wo = np.asarray(Wo, np.float32)
    bq = np.asarray(bq, np.float32)
    bk = np.asarray(bk, np.float32)
    bv = np.asarray(bv, np.float32)
    bo = np.asarray(bo, np.float32)
    def ptile(m, kt):
        # [kt*128, cols] -> [128, kt*cols]: row p holds partition p's
        # SBUF data contiguously (k-tile-major), matching the kernel's
        # on-chip layout so each DMA descriptor is one long run
        cols = m.shape[1]
        return np.ascontiguousarray(
            m.reshape(kt, 128, cols).transpose(1, 0, 2).reshape(128, -1)
            .astype(np.float16))

    xT = {}
    for b in range(B):
        xT[("q", b)] = ptile(inputs_q[b].T, KT)
        xT[("kv", b)] = ptile(inputs_kv[b].T, KT)
    shared = {
        "ones_mat": np.ones((128, 128), np.float16),
    }
    in_maps = []
    for c in range(NCORES):
        b, t = divmod(c, G)
        hsl = slice(2 * t * DH, 2 * t * DH + HC)
        bvs = np.tile(bv[hsl], 2)
        in_maps.append({
            "xqT": xT[("q", b)],
            "xkvT": xT[("kv", b)],
            "wq": ptile(Wq[:, hsl], KT),
            "wk": ptile(Wk[:, hsl], KT),
            "wv": ptile(Wv[:, hsl], KT),
            "wo": ptile(Wo[hsl, :], HL),
            "bq_col": np.ascontiguousarray(bq[hsl].reshape(HL, 128).T),
            "bk_col": np.ascontiguousarray(bk[hsl].reshape(HL, 128).T),
            "bv_row": np.ascontiguousarray(np.broadcast_to(bvs, (128, 2 * HC))),
            **shared,
        })
    return in_maps


def kernel(inputs_q, inputs_kv, Wq, bq, Wk, bk, Wv, bv, Wo, bo):
    in_maps = prep_in_maps(inputs_q, inputs_kv, Wq, bq, Wk, bk, Wv, bv, Wo, bo)
    nc = build_nc(reps=1)
    res = run_bass_kernel_spmd(nc, in_maps, core_ids=list(range(NCORES)))
    out = np.empty((B, N, C), np.float32)
    outv = out.reshape(B, G, 4, G, 32, C)  # [b, tq, tb4, t, pl, c]
    for c in range(NCORES):
        b, t = divmod(c, G)
        o = res.results[c]["out"].astype(np.float32)
        # o[pl, (tq, tb4, c)] -> tokens tq*512 + tb4*128 + t*32 + pl
        outv[b, :, :, t] = o.reshape(32, G, 4, C).transpose(1, 2, 0, 3)
    out += np.asarray(bo, np.float32)  # bo applied host-side
    return out


if __name__ == "__main__":
    rng = np.random.default_rng(0)
    s = 1.0 / np.sqrt(C)
    ins = {
        "inputs_q": rng.standard_normal((B, N, C), np.float32),
        "inputs_kv": rng.standard_normal((B, N, C), np.float32),
        "Wq": rng.standard_normal((C, C), np.float32) * s,
        "bq": np.zeros(C, np.float32),
        "Wk": rng.standard_normal((C, C), np.float32) * s,
        "bk": np.zeros(C, np.float32),
        "Wv": rng.standard_normal((C, C), np.float32) * s,
        "bv": np.zeros(C, np.float32),
        "Wo": rng.standard_normal((C, C), np.float32) * s,
        "bo": np.zeros(C, np.float32),
    }
    out = kernel(**ins)
    # numpy reference
    def ref(xq, xkv, Wq, bq, Wk, bk, Wv, bv, Wo, bo):
        q = (xq @ Wq + bq).reshape(B, N, H, DH)
        k = (xkv @ Wk + bk).reshape(B, N, H, DH)
        v = (xkv @ Wv + bv).reshape(B, N, H, DH)
        s_ = np.einsum("bnhc,bmhc->bhnm", q, k) / np.sqrt(DH)
        e = np.exp(s_ - s_.max(-1, keepdims=True))
        p = e / e.sum(-1, keepdims=True)
        o = np.einsum("bhnm,bmhd->bnhd", p, v).reshape(B, N, C)
        return o @ Wo + bo
    exp = ref(ins["inputs_q"], ins["inputs_kv"], ins["Wq"], ins["bq"],
              ins["Wk"], ins["bk"], ins["Wv"], ins["bv"], ins["Wo"],
              ins["bo"])
    err = np.abs(out - exp).max() / np.abs(exp).max()
    print("out", out.shape, out.dtype, "rel err:", err)

